# revision 6
# baseline (speedup 1.0000x reference)
"""Trainium2 Bass kernel for nn_FFWRelativeCrossAttentionModule.

Sharding: 8 cores = (batch b in 0..3) x (query half qh in 0..1).
Attention mixes query<->value tokens only, so this split is exactly
communication-free: each core runs all 4 layers for its 512 queries of
its batch element against the full 2048 kv positions.

Per-core layout: activations are feature-major [C(part-chunks), tokens];
rotary is applied during PSUM eviction as x*cos + x2*sin where
x2 = value @ rot2(W).T uses host-permuted weights (an extra matmul
instead of partition shuffles). V is projected token-major (stationary
operand = value.T) with a per-head ones column so attn@v also yields the
softmax denominator; 1/x is computed as exp(-ln(x)) (same ACT table
set as the softmax exp). Per-token LN stats come from scaled-ones
matmuls; per-token broadcasts use indicator matmuls in float32r.
"""

import os
import sys

for _p in ("/opt/trn_rl_repo", "/root/.axon_site/_ro/trn_rl_repo"):
    if os.path.isdir(_p) and _p not in sys.path:
        sys.path.append(_p)

import numpy as np
import ml_dtypes

L, C, H, HD = 4, 512, 8, 64
NT = 512          # query tokens per core
NK = 2048         # kv tokens
CH = C // 128     # 4 chunks of 128 channels
SCALING = HD ** -0.5
BF = ml_dtypes.bfloat16

_CACHE = {}


def _rot2_rows(w):
    # rot2(x)[2i] = -x[2i+1]; rot2(x)[2i+1] = x[2i], applied to the
    # projection output channels = rows of w.
    w2 = np.empty_like(w)
    w2[0::2] = -w[1::2]
    w2[1::2] = w[0::2]
    return w2


def _pack_w(m):
    # [C_in, C_out] -> [128, CH(kc), C_out] lhsT tile layout
    return np.ascontiguousarray(np.transpose(m.reshape(CH, 128, C), (1, 0, 2)))


def _pack_fm(m):
    # feature-major [C, N] -> [128, CH, N]
    n = m.shape[1]
    return np.ascontiguousarray(np.transpose(m.reshape(CH, 128, n), (1, 0, 2)))


def _silu(x):
    return x / (1.0 + np.exp(-x))


def _build(flags):
    import concourse.bass as bass
    import concourse.mybir as mybir
    import concourse.tile as tile
    from concourse import bacc

    dt = mybir.dt
    AF = mybir.ActivationFunctionType
    AO = mybir.AluOpType

    has_in_b, has_out_b, has_b1, has_b2, has_ln_g, has_ln_b = flags

    nc = bacc.Bacc("TRN2", target_bir_lowering=False, debug=False, num_devices=8)

    d_qT = nc.declare_dram_parameter("qT", [128, CH, NT], dt.float32, isOutput=False)
    d_vT = nc.declare_dram_parameter("vT", [128, CH, NK], dt.bfloat16, isOutput=False)
    d_cq = nc.declare_dram_parameter("cq", [128, CH, NT], dt.bfloat16, isOutput=False)
    d_sq = nc.declare_dram_parameter("sq", [128, CH, NT], dt.bfloat16, isOutput=False)
    d_ck = nc.declare_dram_parameter("ck", [128, CH, NK], dt.bfloat16, isOutput=False)
    d_sk = nc.declare_dram_parameter("sk", [128, CH, NK], dt.bfloat16, isOutput=False)
    d_wts = nc.declare_dram_parameter("wts", [L, 8, 128, CH, C], dt.bfloat16, isOutput=False)
    d_ada = nc.declare_dram_parameter("ada", [128, L * 4 * CH], dt.float32, isOutput=False)
    d_ind8 = nc.declare_dram_parameter("ind8", [8, C], dt.float32r, isOutput=False)
    d_ones1 = nc.declare_dram_parameter("ones1", [1, 128], dt.float32r, isOutput=False)
    d_bias = nc.declare_dram_parameter("bias", [128, L * 7 * CH], dt.float32, isOutput=False)
    d_lng = nc.declare_dram_parameter("lng", [128, L * 4 * CH], dt.float32, isOutput=False)
    d_out = nc.declare_dram_parameter("out", [L, 128, CH, NT], dt.float32, isOutput=True)

    W_NAMES = ["wq", "wq2", "wk", "wk2", "wv", "wo", "w1", "w2"]

    def ada_col(i, qty, c):
        return (i * 4 + qty) * CH + c

    def bias_col(i, qty, c):
        return (i * 7 + qty) * CH + c

    def lng_col(i, qty, c):
        return (i * 4 + qty) * CH + c

    with tile.TileContext(nc) as tc:
        with tc.tile_pool(name="const", bufs=1) as cpool, \
             tc.tile_pool(name="wpool", bufs=1) as wpool, \
             tc.tile_pool(name="state", bufs=1) as spool, \
             tc.tile_pool(name="epool", bufs=2) as epool, \
             tc.tile_pool(name="act", bufs=3) as apool, \
             tc.tile_pool(name="resid", bufs=2) as rpool, \
             tc.tile_pool(name="scr", bufs=3) as scrpool, \
             tc.tile_pool(name="scrf", bufs=2) as scfpool, \
             tc.tile_pool(name="small", bufs=2) as smpool, \
             tc.tile_pool(name="stats", bufs=3) as stpool, \
             tc.tile_pool(name="psum", bufs=2, space="PSUM") as ppool:

            # ---- resident constants ----
            vT = cpool.tile([128, CH, NK], dt.bfloat16)
            nc.gpsimd.dma_start(vT[:], d_vT[:])
            ck = cpool.tile([128, CH, NK], dt.bfloat16)
            nc.gpsimd.dma_start(ck[:], d_ck[:])
            sk = cpool.tile([128, CH, NK], dt.bfloat16)
            nc.gpsimd.dma_start(sk[:], d_sk[:])
            cq = cpool.tile([128, CH, NT], dt.bfloat16)
            nc.gpsimd.dma_start(cq[:], d_cq[:])
            sq = cpool.tile([128, CH, NT], dt.bfloat16)
            nc.gpsimd.dma_start(sq[:], d_sq[:])
            ada = cpool.tile([128, L * 4 * CH], dt.float32)
            nc.gpsimd.dma_start(ada[:], d_ada[:])
            ind8 = cpool.tile([8, C], dt.float32r)
            nc.gpsimd.dma_start(ind8[:], d_ind8[:])
            if has_in_b or has_out_b or has_b1 or has_b2:
                bias = cpool.tile([128, L * 7 * CH], dt.float32)
                nc.gpsimd.dma_start(bias[:], d_bias[:])
            if has_ln_g or has_ln_b:
                lng = cpool.tile([128, L * 4 * CH], dt.float32)
                nc.gpsimd.dma_start(lng[:], d_lng[:])
            ones_sc = cpool.tile([128, 1], dt.bfloat16)   # 1/C for mean matmuls
            nc.vector.memset(ones_sc[:], 1.0 / C)
            ones1 = cpool.tile([1, 128], dt.float32r)      # broadcast lhsT
            nc.gpsimd.dma_start(ones1[:], d_ones1[:])

            # ---- persistent state ----
            q = spool.tile([128, CH, NT], dt.float32)     # master q (feature-major)
            nc.gpsimd.dma_start(q[:], d_qT[:])
            kb = spool.tile([128, CH, NK], dt.bfloat16)   # rotary-embedded K.T
            vb = spool.tile([128, NK // 128, H, HD + 1], dt.bfloat16)  # token-major V + ones col
            nc.vector.memset(vb[:, :, :, HD:HD + 1], 1.0)

            def b_ap(i, qty, c):
                # per-partition bias scalar or 0.0 immediate
                return bias[:, bias_col(i, qty, c):bias_col(i, qty, c) + 1]

            def layer(i):
                w = {}
                for j, nm in enumerate(W_NAMES):
                    t = wpool.tile([128, CH, C], dt.bfloat16, tag=f"w{j}", name=f"w_{nm}_{i}")
                    nc.gpsimd.dma_start(t[:], d_wts[i, j])
                    w[nm] = t

                # ---- adaln (attn) ----
                xa = apool.tile([128, CH, NT], dt.bfloat16, tag="act", name=f"xa_{i}")
                for c in range(CH):
                    nc.vector.tensor_scalar(
                        xa[:, c, :], q[:, c, :],
                        ada[:, ada_col(i, 0, c):ada_col(i, 0, c) + 1],
                        ada[:, ada_col(i, 1, c):ada_col(i, 1, c) + 1],
                        AO.mult, AO.add)

                # ---- q projection + rotary -> qb ----
                qb = spool.tile([128, CH, NT], dt.bfloat16, tag="qb", name=f"qb_{i}")
                for mc in range(CH):
                    pq = ppool.tile([128, 512], dt.float32, tag="mm", name=f"pq_{i}_{mc}")
                    for kc in range(CH):
                        nc.tensor.matmul(pq[:], w["wq"][:, kc, mc * 128:(mc + 1) * 128],
                                         xa[:, kc, :], start=(kc == 0), stop=(kc == CH - 1))
                    pq2 = ppool.tile([128, 512], dt.float32, tag="mm", name=f"pq2_{i}_{mc}")
                    for kc in range(CH):
                        nc.tensor.matmul(pq2[:], w["wq2"][:, kc, mc * 128:(mc + 1) * 128],
                                         xa[:, kc, :], start=(kc == 0), stop=(kc == CH - 1))
                    t1 = scrpool.tile([128, 512], dt.bfloat16, tag="scr", name=f"qc_{i}_{mc}")
                    nc.vector.scalar_tensor_tensor(
                        t1[:], pq[:], b_ap(i, 0, mc) if has_in_b else 0.0,
                        cq[:, mc, :], AO.add, AO.mult)
                    t2 = scrpool.tile([128, 512], dt.bfloat16, tag="scr", name=f"qs_{i}_{mc}")
                    nc.vector.scalar_tensor_tensor(
                        t2[:], pq2[:], b_ap(i, 1, mc) if has_in_b else 0.0,
                        sq[:, mc, :], AO.add, AO.mult)
                    nc.vector.tensor_add(qb[:, mc, :], t1[:], t2[:])

                # ---- k projection + rotary -> kb ; v projection -> vb ----
                for n in range(CH):          # kv 512-col groups
                    ksl = slice(n * 512, (n + 1) * 512)
                    for mc in range(CH):
                        pk = ppool.tile([128, 512], dt.float32, tag="mm", name=f"pk_{i}_{n}_{mc}")
                        for kc in range(CH):
                            nc.tensor.matmul(pk[:], w["wk"][:, kc, mc * 128:(mc + 1) * 128],
                                             vT[:, kc, ksl], start=(kc == 0), stop=(kc == CH - 1))
                        pk2 = ppool.tile([128, 512], dt.float32, tag="mm", name=f"pk2_{i}_{n}_{mc}")
                        for kc in range(CH):
                            nc.tensor.matmul(pk2[:], w["wk2"][:, kc, mc * 128:(mc + 1) * 128],
                                             vT[:, kc, ksl], start=(kc == 0), stop=(kc == CH - 1))
                        t1 = scrpool.tile([128, 512], dt.bfloat16, tag="scr", name=f"kc_{i}_{n}_{mc}")
                        nc.vector.scalar_tensor_tensor(
                            t1[:], pk[:], b_ap(i, 2, mc) if has_in_b else 0.0,
                            ck[:, mc, ksl], AO.add, AO.mult)
                        t2 = scrpool.tile([128, 512], dt.bfloat16, tag="scr", name=f"ks_{i}_{n}_{mc}")
                        nc.vector.scalar_tensor_tensor(
                            t2[:], pk2[:], b_ap(i, 3, mc) if has_in_b else 0.0,
                            sk[:, mc, ksl], AO.add, AO.mult)
                        nc.vector.tensor_add(kb[:, mc, ksl], t1[:], t2[:])
                    # v token-major: stationary = value.T chunk, moving = wv
                    for t in range(4):
                        idx = n * 4 + t
                        pv = ppool.tile([128, 512], dt.float32, tag="mm", name=f"pv_{i}_{idx}")
                        for kc in range(CH):
                            nc.tensor.matmul(pv[:], vT[:, kc, idx * 128:(idx + 1) * 128],
                                             w["wv"][:, kc, :], start=(kc == 0), stop=(kc == CH - 1))
                        nc.vector.tensor_copy(
                            vb[:, idx, :, 0:HD],
                            pv[:].rearrange("p (h d) -> p h d", h=H))

                # ---- attention ----
                o_raw = apool.tile([128, CH, NT], dt.bfloat16, tag="act", name=f"oraw_{i}")
                den_all = smpool.tile([8, 512], dt.float32, tag="den", name=f"den_{i}")
                for h in range(H):
                    qc0, off = h // 2, (h % 2) * 64
                    Eh = [epool.tile([128, 8, 512], dt.bfloat16, tag="E", name=f"E_{i}_{h}_{u}")
                          for u in range(2)]
                    oacc = ppool.tile([128, 512], dt.float32, tag="acc", name=f"oacc_{i}_{h}")
                    for g in range(8):
                        ps = ppool.tile([128, 2, 512], dt.float32, tag="sc", name=f"ps_{i}_{h}_{g}")
                        for j in range(2):
                            nidx = g * 2 + j
                            nc.tensor.matmul(
                                ps[:, j, :],
                                kb[off:off + 64, qc0, nidx * 128:(nidx + 1) * 128],
                                qb[off:off + 64, qc0, :], start=True, stop=True)
                        E = Eh[g // 4]
                        gg = g % 4
                        nc.scalar.activation(E[:, gg * 2:gg * 2 + 2, :], ps[:], AF.Exp)
                    for nidx in range(16):
                        nc.tensor.matmul(oacc[0:HD + 1, :], vb[:, nidx, h, :],
                                         Eh[nidx // 8][:, nidx % 8, :],
                                         start=(nidx == 0), stop=(nidx == 15))
                    nc.vector.tensor_copy(o_raw[off:off + 64, qc0, :], oacc[0:HD, :])
                    dh = smpool.tile([1, 512], dt.float32, tag="denh", name=f"dh_{i}_{h}")
                    nc.vector.tensor_copy(dh[:], oacc[HD:HD + 1, :])
                    nc.gpsimd.dma_start(den_all[h:h + 1, :], dh[:])
                # 1/den = exp(-ln(den)); broadcast per head-pair via indicator matmul
                lnd = smpool.tile([8, 512], dt.float32, tag="den2", name=f"lnd_{i}")
                nc.scalar.activation(lnd[:], den_all[:], AF.Ln)
                recip = smpool.tile([8, 512], dt.float32r, tag="den2", name=f"rc_{i}")
                nc.scalar.activation(recip[:], lnd[:], AF.Exp, scale=-1.0)
                o_norm = apool.tile([128, CH, NT], dt.bfloat16, tag="act", name=f"onrm_{i}")
                for pc in range(CH):
                    rb = ppool.tile([128, 512], dt.float32, tag="mm", name=f"rb_{i}_{pc}")
                    nc.tensor.matmul(rb[:], ind8[:, pc * 128:(pc + 1) * 128],
                                     recip[:], start=True, stop=True)
                    nc.vector.tensor_mul(o_norm[:, pc, :], o_raw[:, pc, :], rb[:])

                # ---- out projection + residual ----
                r1 = rpool.tile([128, CH, NT], dt.float32, tag="resid", name=f"r1_{i}")
                for mo in range(CH):
                    po = ppool.tile([128, 512], dt.float32, tag="mm", name=f"po_{i}_{mo}")
                    for kc in range(CH):
                        nc.tensor.matmul(po[:], w["wo"][:, kc, mo * 128:(mo + 1) * 128],
                                         o_norm[:, kc, :], start=(kc == 0), stop=(kc == CH - 1))
                    nc.vector.scalar_tensor_tensor(
                        r1[:, mo, :], po[:], b_ap(i, 4, mo) if has_out_b else 0.0,
                        q[:, mo, :], AO.add, AO.add)

                # ---- LN1 -> q ----
                _ln(i, 0, r1, q)

                # ---- adaln (ffn) ----
                xf = rpool.tile([128, CH, NT], dt.float32, tag="resid", name=f"xf_{i}")
                xfb = apool.tile([128, CH, NT], dt.bfloat16, tag="act", name=f"xfb_{i}")
                for c in range(CH):
                    nc.vector.tensor_scalar(
                        xf[:, c, :], q[:, c, :],
                        ada[:, ada_col(i, 2, c):ada_col(i, 2, c) + 1],
                        ada[:, ada_col(i, 3, c):ada_col(i, 3, c) + 1],
                        AO.mult, AO.add)
                    nc.vector.tensor_copy(xfb[:, c, :], xf[:, c, :])

                # ---- FFN ----
                hbf = apool.tile([128, CH, NT], dt.bfloat16, tag="act", name=f"hbf_{i}")
                for mh in range(CH):
                    ph = ppool.tile([128, 512], dt.float32, tag="mm", name=f"ph_{i}_{mh}")
                    for kc in range(CH):
                        nc.tensor.matmul(ph[:], w["w1"][:, kc, mh * 128:(mh + 1) * 128],
                                         xfb[:, kc, :], start=(kc == 0), stop=(kc == CH - 1))
                    nc.vector.tensor_scalar(
                        hbf[:, mh, :], ph[:],
                        b_ap(i, 5, mh) if has_b1 else 0.0, 0.0, AO.add, AO.max)
                r2 = rpool.tile([128, CH, NT], dt.float32, tag="resid", name=f"r2_{i}")
                for mo in range(CH):
                    pf = ppool.tile([128, 512], dt.float32, tag="mm", name=f"pf_{i}_{mo}")
                    for kc in range(CH):
                        nc.tensor.matmul(pf[:], w["w2"][:, kc, mo * 128:(mo + 1) * 128],
                                         hbf[:, kc, :], start=(kc == 0), stop=(kc == CH - 1))
                    nc.vector.scalar_tensor_tensor(
                        r2[:, mo, :], pf[:], b_ap(i, 6, mo) if has_b2 else 0.0,
                        xf[:, mo, :], AO.add, AO.add)

                # ---- LN2 -> q ; emit layer output ----
                _ln(i, 1, r2, q)
                for c in range(CH):
                    nc.gpsimd.dma_start(d_out[i, :, c, :], q[:, c, :])

            def _ln(i, which, rin, qout):
                # stats from bf16 copy of rin
                rbf = apool.tile([128, CH, NT], dt.bfloat16, tag="act", name=f"rbf_{i}_{which}")
                r2b = apool.tile([128, CH, NT], dt.bfloat16, tag="act", name=f"r2b_{i}_{which}")
                for c in range(CH):
                    nc.vector.tensor_copy(rbf[:, c, :], rin[:, c, :])
                    nc.scalar.activation(r2b[:, c, :], rbf[:, c, :], AF.Square)
                pm = ppool.tile([128, 512], dt.float32, tag="acc", name=f"pm_{i}_{which}")
                pv = ppool.tile([128, 512], dt.float32, tag="acc", name=f"pv2_{i}_{which}")
                for c in range(CH):
                    nc.tensor.matmul(pm[0:1, :], ones_sc[:], rbf[:, c, :],
                                     start=(c == 0), stop=(c == CH - 1))
                for c in range(CH):
                    nc.tensor.matmul(pv[0:1, :], ones_sc[:], r2b[:, c, :],
                                     start=(c == 0), stop=(c == CH - 1))
                m_sb = stpool.tile([1, 512], dt.float32r, tag="st", name=f"m_{i}_{which}")
                nc.vector.tensor_copy(m_sb[:], pm[0:1, :])
                msq = stpool.tile([1, 512], dt.float32, tag="st", name=f"msq_{i}_{which}")
                nc.vector.tensor_mul(msq[:], m_sb[:], m_sb[:])
                var = stpool.tile([1, 512], dt.float32, tag="st", name=f"var_{i}_{which}")
                nc.vector.scalar_tensor_tensor(var[:], pv[0:1, :], 1e-5, msq[:],
                                               AO.add, AO.subtract)
                lnv = stpool.tile([1, 512], dt.float32, tag="st", name=f"lnv_{i}_{which}")
                nc.scalar.activation(lnv[:], var[:], AF.Ln)
                rstd = stpool.tile([1, 512], dt.float32r, tag="st", name=f"rstd_{i}_{which}")
                nc.scalar.activation(rstd[:], lnv[:], AF.Exp, scale=-0.5)
                mb = ppool.tile([128, 512], dt.float32, tag="mm", name=f"mb_{i}_{which}")
                nc.tensor.matmul(mb[:], ones1[:], m_sb[:], start=True, stop=True)
                rsb = ppool.tile([128, 512], dt.float32, tag="mm", name=f"rsb_{i}_{which}")
                nc.tensor.matmul(rsb[:], ones1[:], rstd[:], start=True, stop=True)
                for c in range(CH):
                    t1 = scfpool.tile([128, 512], dt.float32, tag="scf", name=f"lt_{i}_{which}_{c}")
                    nc.vector.tensor_sub(t1[:], rin[:, c, :], mb[:])
                    g_arg = (lng[:, lng_col(i, which * 2, c):lng_col(i, which * 2, c) + 1]
                             if has_ln_g else 1.0)
                    nc.vector.scalar_tensor_tensor(qout[:, c, :], t1[:], g_arg, rsb[:],
                                                   AO.mult, AO.mult)
                    if has_ln_b:
                        nc.vector.tensor_scalar_add(
                            qout[:, c, :], qout[:, c, :],
                            lng[:, lng_col(i, which * 2 + 1, c):lng_col(i, which * 2 + 1, c) + 1])

            for i in range(L):
                layer(i)

    nc.compile()
    return nc


def _prep_core(inputs, core, host):
    b, qh = core // 2, core % 2
    sl = slice(qh * NT, (qh + 1) * NT)
    im = {
        "qT": np.ascontiguousarray(_pack_fm(inputs["query"][sl, b, :].T.astype(np.float32))),
        "vT": _pack_fm(inputs["value"][:, b, :].T).astype(BF),
        "cq": _pack_fm(inputs["query_pos"][b, sl, :, 0].T).astype(BF),
        "sq": _pack_fm(inputs["query_pos"][b, sl, :, 1].T).astype(BF),
        "ck": _pack_fm(inputs["value_pos"][b, :, :, 0].T).astype(BF),
        "sk": _pack_fm(inputs["value_pos"][b, :, :, 1].T).astype(BF),
        "wts": host["wts"],
        "ada": host["ada"][b],
        "ind8": host["ind8"],
        "ones1": np.ones((1, 128), np.float32),
        "bias": host["bias"],
        "lng": host["lng"],
    }
    return im


def _prep_host(inputs):
    wts = np.zeros((L, 8, 128, CH, C), BF)
    bias = np.zeros((128, L * 7 * CH), np.float32)
    lng = np.zeros((128, L * 4 * CH), np.float32)
    for i in range(L):
        in_w, in_b = np.asarray(inputs["in_w"][i]), np.asarray(inputs["in_b"][i])
        wq = in_w[:C] * SCALING
        wk, wv = in_w[C:2 * C], in_w[2 * C:]
        bq = in_b[:C] * SCALING
        bk, bv = in_b[C:2 * C], in_b[2 * C:]
        if np.any(bv):
            raise NotImplementedError("nonzero v-projection bias not supported")
        mats = [wq.T, _rot2_rows(wq).T, wk.T, _rot2_rows(wk).T, wv.T,
                np.asarray(inputs["out_w"][i]).T,
                np.asarray(inputs["w1"][i]).T, np.asarray(inputs["w2"][i]).T]
        for j, m in enumerate(mats):
            wts[i, j] = _pack_w(np.ascontiguousarray(m)).astype(BF)
        bvecs = [bq, _rot2_rows(bq.reshape(-1, 1)).ravel(), bk,
                 _rot2_rows(bk.reshape(-1, 1)).ravel(),
                 np.asarray(inputs["out_b"][i]), np.asarray(inputs["b1"][i]),
                 np.asarray(inputs["b2"][i])]
        for qy, v in enumerate(bvecs):
            for c in range(CH):
                bias[:, (i * 7 + qy) * CH + c] = v[c * 128:(c + 1) * 128]
        lvecs = [np.asarray(inputs["ln1_g"][i]), np.asarray(inputs["ln1_b"][i]),
                 np.asarray(inputs["ln2_g"][i]), np.asarray(inputs["ln2_b"][i])]
        for qy, v in enumerate(lvecs):
            for c in range(CH):
                lng[:, (i * 4 + qy) * CH + c] = v[c * 128:(c + 1) * 128]

    ada = np.zeros((4, 128, L * 4 * CH), np.float32)
    diff = np.asarray(inputs["diff_ts"], np.float32)
    for b in range(4):
        st = _silu(diff[b])
        for i in range(L):
            for qy, (aw, ab) in enumerate(
                    [(inputs["aw_attn"][i], inputs["ab_attn"][i]),
                     (inputs["aw_ffn"][i], inputs["ab_ffn"][i])]):
                mod = st @ np.asarray(aw, np.float32).T + np.asarray(ab, np.float32)
                sc, sh = 1.0 + mod[:C], mod[C:]
                for c in range(CH):
                    ada[b, :, (i * 4 + 2 * qy) * CH + c] = sc[c * 128:(c + 1) * 128]
                    ada[b, :, (i * 4 + 2 * qy + 1) * CH + c] = sh[c * 128:(c + 1) * 128]

    ind8 = np.zeros((8, C), np.float32)
    for h in range(H):
        base = (h // 2) * 128 + (h % 2) * 64
        ind8[h, base:base + 64] = 1.0

    flags = (bool(np.any(np.asarray(inputs["in_b"]))),
             bool(np.any(np.asarray(inputs["out_b"]))),
             bool(np.any(np.asarray(inputs["b1"]))),
             bool(np.any(np.asarray(inputs["b2"]))),
             bool(np.any(np.asarray(inputs["ln1_g"]) != 1.0) or np.any(np.asarray(inputs["ln2_g"]) != 1.0)),
             bool(np.any(np.asarray(inputs["ln1_b"])) or np.any(np.asarray(inputs["ln2_b"]))))
    return dict(wts=wts, ada=ada, ind8=ind8, bias=bias, lng=lng), flags


def _get_program(flags):
    if flags not in _CACHE:
        _CACHE[flags] = _build(flags)
    return _CACHE[flags]


def _assemble(results):
    full = np.zeros((L, 1024, 4, C), np.float32)
    for core in range(8):
        b, qh = core // 2, core % 2
        arr = results[core]["out"]                     # [L, 128, CH, NT]
        fm = np.transpose(arr, (0, 2, 1, 3)).reshape(L, C, NT)
        full[:, qh * NT:(qh + 1) * NT, b, :] = np.transpose(fm, (0, 2, 1))
    return full


def kernel(**inputs):
    from concourse.bass_utils import run_bass_kernel_spmd

    inputs = {k: np.asarray(v) for k, v in inputs.items()}
    host, flags = _prep_host(inputs)
    nc = _get_program(flags)
    in_maps = [_prep_core(inputs, core, host) for core in range(8)]
    res = run_bass_kernel_spmd(nc, in_maps, list(range(8)))
    return _assemble(res.results)


# revision 22
# speedup vs baseline: 5315.8260x; 5315.8260x over previous
"""Trainium2 Bass kernel for nn_FFWRelativeCrossAttentionModule.

Sharding: 8 cores = (batch b in 0..3) x (query half qh in 0..1).
Attention mixes query<->value tokens only, so this split is exactly
communication-free: each core runs all 4 layers for its 512 queries of
its batch element against the full 2048 kv positions.

Per-core layout: activations are feature-major [C(part-chunks), tokens];
rotary is applied during PSUM eviction as x*cos + x2*sin where
x2 = value @ rot2(W).T uses host-permuted weights (an extra matmul
instead of partition shuffles). V is projected token-major (stationary
operand = value.T) with a per-head ones column so attn@v also yields the
softmax denominator; 1/x is computed as exp(-ln(x)) (same ACT table
set as the softmax exp). Per-token LN stats come from scaled-ones
matmuls; per-token broadcasts use indicator matmuls in float32r.
"""

import os
import sys

for _p in ("/opt/trn_rl_repo", "/root/.axon_site/_ro/trn_rl_repo"):
    if os.path.isdir(_p) and _p not in sys.path:
        sys.path.append(_p)

import numpy as np
import ml_dtypes

L, C, H, HD = 4, 512, 8, 64
NT = 512          # query tokens per core
NK = 2048         # kv tokens
CH = C // 128     # 4 chunks of 128 channels
SCALING = HD ** -0.5
BF = ml_dtypes.bfloat16

_CACHE = {}


def _rot2_rows(w):
    # rot2(x)[2i] = -x[2i+1]; rot2(x)[2i+1] = x[2i], applied to the
    # projection output channels = rows of w.
    w2 = np.empty_like(w)
    w2[0::2] = -w[1::2]
    w2[1::2] = w[0::2]
    return w2


def _pack_w(m):
    # [C_in, C_out] -> [128, CH(kc), C_out] lhsT tile layout
    return np.ascontiguousarray(np.transpose(m.reshape(CH, 128, C), (1, 0, 2)))


def _pack_fm(m):
    # feature-major [C, N] -> [128, CH, N]
    n = m.shape[1]
    return np.ascontiguousarray(np.transpose(m.reshape(CH, 128, n), (1, 0, 2)))


def _silu(x):
    return x / (1.0 + np.exp(-x))


def _pin_act_tables():
    """Make every activation resolve to natural_log_exp_and_others so the
    kernel uses one ACT table set (no ~1.3us reloads between Exp/Ln)."""
    from concourse import bacc as _bacc
    from concourse.hw_specs import get_activation_tables as _orig

    def patched(arch):
        tabs = _orig(arch)
        keep = "natural_log_exp_and_others"
        if keep in tabs:
            tabs = {k: (v if k == keep else set()) for k, v in tabs.items()}
        return tabs

    _bacc.get_activation_tables = patched


def _build(flags, nrep=1, unroll=1):
    import concourse.bass as bass
    import concourse.mybir as mybir
    import concourse.tile as tile
    from concourse import bacc

    _pin_act_tables()

    dt = mybir.dt
    AF = mybir.ActivationFunctionType
    AO = mybir.AluOpType

    has_in_b, has_out_b, has_b1, has_b2, has_ln_g, has_ln_b = flags

    nc = bacc.Bacc("TRN2", target_bir_lowering=False, debug=False, num_devices=8)

    d_qT = nc.declare_dram_parameter("qT", [128, CH, NT], dt.float32, isOutput=False)
    d_vT = nc.declare_dram_parameter("vT", [128, CH, NK], dt.bfloat16, isOutput=False)
    d_cq = nc.declare_dram_parameter("cq", [128, CH, NT], dt.bfloat16, isOutput=False)
    d_sq = nc.declare_dram_parameter("sq", [128, CH, NT], dt.bfloat16, isOutput=False)
    d_ck = nc.declare_dram_parameter("ck", [128, CH, NK], dt.bfloat16, isOutput=False)
    d_sk = nc.declare_dram_parameter("sk", [128, CH, NK], dt.bfloat16, isOutput=False)
    d_wts = nc.declare_dram_parameter("wts", [L, 8, 128, CH, C], dt.bfloat16, isOutput=False)
    d_ada = nc.declare_dram_parameter("ada", [128, L * 4 * CH], dt.float32, isOutput=False)
    d_ind8 = nc.declare_dram_parameter("ind8", [8, C], dt.float32r, isOutput=False)
    d_ones1 = nc.declare_dram_parameter("ones1", [1, 128], dt.float32r, isOutput=False)
    d_ind2 = nc.declare_dram_parameter("ind2", [2, 128], dt.float32r, isOutput=False)
    d_bias = nc.declare_dram_parameter("bias", [128, L * 7 * CH], dt.float32, isOutput=False)
    d_lng = nc.declare_dram_parameter("lng", [128, L * 4 * CH], dt.float32, isOutput=False)
    d_out = nc.declare_dram_parameter("out", [L, 128, CH, NT], dt.float32, isOutput=True)

    SWAP_MASK = [j + 1 if j % 2 == 0 else j - 1 for j in range(32)]
    W_NAMES = ["wk", "wv", "wq", "wo", "w1", "w2"]

    def ada_col(i, qty, c):
        return (i * 4 + qty) * CH + c

    def bias_col(i, qty, c):
        return (i * 7 + qty) * CH + c

    def lng_col(i, qty, c):
        return (i * 4 + qty) * CH + c

    with tile.TileContext(nc) as tc:
        with tc.tile_pool(name="const", bufs=1) as cpool, \
             tc.tile_pool(name="wpool", bufs=1) as wpool, \
             tc.tile_pool(name="state", bufs=1) as spool, \
             tc.tile_pool(name="epool", bufs=4) as epool, \
             tc.tile_pool(name="act", bufs=3) as apool, \
             tc.tile_pool(name="resid", bufs=2) as rpool, \
             tc.tile_pool(name="scr", bufs=3) as scrpool, \
             tc.tile_pool(name="scrf", bufs=2) as scfpool, \
             tc.tile_pool(name="cskpool", bufs=2) as cskpool, \
             tc.tile_pool(name="small", bufs=2) as smpool, \
             tc.tile_pool(name="stats", bufs=3) as stpool, \
             tc.tile_pool(name="psum", bufs=2, space="PSUM") as ppool:

            # ---- resident constants (small/urgent first; ck/sk trickle) ----
            ada = cpool.tile([128, L * 4 * CH], dt.float32)
            nc.sync.dma_start(ada[:], d_ada[:])
            ind8 = cpool.tile([8, C], dt.float32r)
            nc.sync.dma_start(ind8[:], d_ind8[:])
            cq = cpool.tile([128, CH, NT], dt.bfloat16)
            nc.sync.dma_start(cq[:], d_cq[:])
            sq = cpool.tile([128, CH, NT], dt.bfloat16)
            nc.sync.dma_start(sq[:], d_sq[:])
            vT = cpool.tile([128, CH, NK], dt.bfloat16)
            for _c in range(CH):
                eng = nc.gpsimd if _c % 2 == 0 else nc.sync
                eng.dma_start(vT[:, _c, :], d_vT[:, _c, :])
            if has_in_b or has_out_b or has_b1 or has_b2:
                bias = cpool.tile([128, L * 7 * CH], dt.float32)
                nc.sync.dma_start(bias[:], d_bias[:])
            if has_ln_g or has_ln_b:
                lng = cpool.tile([128, L * 4 * CH], dt.float32)
                nc.sync.dma_start(lng[:], d_lng[:])
            ones_sc = cpool.tile([128, 1], dt.bfloat16)   # 1/C for mean matmuls
            nc.vector.memset(ones_sc[:], 1.0 / C)
            ones1 = cpool.tile([1, 128], dt.float32r)      # broadcast lhsT
            nc.sync.dma_start(ones1[:], d_ones1[:])
            ind2 = cpool.tile([2, 128], dt.float32r)       # pair indicator
            nc.sync.dma_start(ind2[:], d_ind2[:])

            # ---- persistent state ----
            q = spool.tile([128, CH, NT], dt.float32)     # master q (feature-major)
            nc.sync.dma_start(q[:], d_qT[:])
            kb = spool.tile([128, CH, NK], dt.bfloat16)   # rotary-embedded K.T
            vb = spool.tile([128, NK // 128, H, HD + 1], dt.bfloat16)  # token-major V + ones col
            nc.vector.memset(vb[:, :, :, HD:HD + 1], 1.0)

            def b_ap(i, qty, c):
                # per-partition bias scalar or 0.0 immediate
                return bias[:, bias_col(i, qty, c):bias_col(i, qty, c) + 1]

            def layer(i):
                w = {}
                W_SLOT = {"wk": 0, "wv": 2, "wq": 3, "wo": 5, "w1": 6, "w2": 7}
                for j, nm in enumerate(W_NAMES):
                    t = wpool.tile([128, CH, C], dt.bfloat16, tag=f"w{j}", name=f"w_{nm}_{i}")
                    nc.gpsimd.dma_start(t[:], d_wts[i, W_SLOT[nm]])
                    w[nm] = t

                # ---- adaln (attn) ----
                xa = apool.tile([128, CH, NT], dt.bfloat16, tag="act", name=f"xa_{i}")
                for c in range(CH):
                    nc.vector.tensor_scalar(
                        xa[:, c, :], q[:, c, :],
                        ada[:, ada_col(i, 0, c):ada_col(i, 0, c) + 1],
                        ada[:, ada_col(i, 1, c):ada_col(i, 1, c) + 1],
                        AO.mult, AO.add)

                # ---- k projection + rotary -> kb ; v projection -> vb ----
                for n in range(CH):          # kv 512-col groups
                    ksl = slice(n * 512, (n + 1) * 512)
                    ckt = cskpool.tile([128, CH, 512], dt.bfloat16, tag="ckt", name=f"ckt_{i}_{n}")
                    nc.sync.dma_start(ckt[:], d_ck[:, :, ksl])
                    skt = cskpool.tile([128, CH, 512], dt.bfloat16, tag="skt", name=f"skt_{i}_{n}")
                    nc.sync.dma_start(skt[:], d_sk[:, :, ksl])
                    for mc in range(CH):
                        pk = ppool.tile([128, 512], dt.float32, tag="kvmm", name=f"pk_{i}_{n}_{mc}")
                        for kc in range(CH):
                            nc.tensor.matmul(pk[:], w["wk"][:, kc, mc * 128:(mc + 1) * 128],
                                             vT[:, kc, ksl], start=(kc == 0), stop=(kc == CH - 1))
                        t1 = scrpool.tile([128, 512], dt.bfloat16, tag="scr", name=f"kc_{i}_{n}_{mc}")
                        nc.vector.scalar_tensor_tensor(
                            t1[:], pk[:], b_ap(i, 2, mc) if has_in_b else 0.0,
                            ckt[:, mc, :], AO.add, AO.mult)
                        ksh = scfpool.tile([128, 512], dt.float32, tag="ksh", name=f"ksh_{i}_{n}_{mc}")
                        nc.vector.stream_shuffle(ksh[:], pk[:], SWAP_MASK)
                        t2 = scrpool.tile([128, 512], dt.bfloat16, tag="scr", name=f"ks_{i}_{n}_{mc}")
                        nc.vector.scalar_tensor_tensor(
                            t2[:], ksh[:], b_ap(i, 3, mc) if has_in_b else 0.0,
                            skt[:, mc, :], AO.add, AO.mult)
                        nc.vector.tensor_add(kb[:, mc, ksl], t1[:], t2[:])
                    # v token-major: stationary = value.T chunk, moving = wv
                    for t in range(4):
                        idx = n * 4 + t
                        pv = ppool.tile([128, 512], dt.float32, tag="kvmm", name=f"pv_{i}_{idx}")
                        for kc in range(CH):
                            nc.tensor.matmul(pv[:], vT[:, kc, idx * 128:(idx + 1) * 128],
                                             w["wv"][:, kc, :], start=(kc == 0), stop=(kc == CH - 1))
                        nc.vector.tensor_copy(
                            vb[:, idx, :, 0:HD],
                            pv[:].rearrange("p (h d) -> p h d", h=H))

                # ---- q projection + rotary -> qb ----
                qb = spool.tile([128, CH, NT], dt.bfloat16, tag="qb", name=f"qb_{i}")
                for mc in range(CH):
                    pq = ppool.tile([128, 512], dt.float32, tag="kvmm", name=f"pq_{i}_{mc}")
                    for kc in range(CH):
                        nc.tensor.matmul(pq[:], w["wq"][:, kc, mc * 128:(mc + 1) * 128],
                                         xa[:, kc, :], start=(kc == 0), stop=(kc == CH - 1))
                    t1 = scrpool.tile([128, 512], dt.bfloat16, tag="scr", name=f"qc_{i}_{mc}")
                    nc.vector.scalar_tensor_tensor(
                        t1[:], pq[:], b_ap(i, 0, mc) if has_in_b else 0.0,
                        cq[:, mc, :], AO.add, AO.mult)
                    qsh = scfpool.tile([128, 512], dt.float32, tag="ksh", name=f"qsh_{i}_{mc}")
                    nc.vector.stream_shuffle(qsh[:], pq[:], SWAP_MASK)
                    t2 = scrpool.tile([128, 512], dt.bfloat16, tag="scr", name=f"qs_{i}_{mc}")
                    nc.vector.scalar_tensor_tensor(
                        t2[:], qsh[:], b_ap(i, 1, mc) if has_in_b else 0.0,
                        sq[:, mc, :], AO.add, AO.mult)
                    nc.vector.tensor_add(qb[:, mc, :], t1[:], t2[:])

                # ---- attention ----
                o_raw = apool.tile([128, CH, NT], dt.bfloat16, tag="act", name=f"oraw_{i}")
                o_norm = apool.tile([128, CH, NT], dt.bfloat16, tag="act", name=f"onrm_{i}")
                den_all = smpool.tile([8, 512], dt.float32, tag="den", name=f"den_{i}")
                for p in range(H // 2):
                    qc0 = p
                    hpair = (2 * p, 2 * p + 1)
                    Eh = {h: [epool.tile([128, 8, 512], dt.bfloat16, tag="E",
                                         name=f"E_{i}_{h}_{u}") for u in range(2)]
                          for h in hpair}
                    # scores: both heads' chunk-n matmuls adjacent -> the PE
                    # runs them concurrently (disjoint row groups, own banks)
                    for g in range(8):
                        ps = {h: ppool.tile([128, 2, 512], dt.float32, tag="sc",
                                            name=f"ps_{i}_{h}_{g}") for h in hpair}
                        for j in range(2):
                            nidx = g * 2 + j
                            for h in hpair:
                                off = (h % 2) * 64
                                nc.tensor.matmul(
                                    ps[h][:, j, :],
                                    kb[off:off + 64, qc0, nidx * 128:(nidx + 1) * 128],
                                    qb[off:off + 64, qc0, :], start=True, stop=True)
                        for h in hpair:
                            E = Eh[h][g // 4]
                            gg = g % 4
                            nc.scalar.activation(E[:, gg * 2:gg * 2 + 2, :], ps[h][:], AF.Exp)
                    for h in hpair:
                        off = (h % 2) * 64
                        oacc = ppool.tile([128, 512], dt.float32, tag="mm",
                                          name=f"oacc_{i}_{h}")
                        for nidx in range(16):
                            nc.tensor.matmul(oacc[0:HD + 1, :], vb[:, nidx, h, :],
                                             Eh[h][nidx // 8][:, nidx % 8, :],
                                             start=(nidx == 0), stop=(nidx == 15))
                        nc.vector.tensor_copy(o_raw[off:off + 64, qc0, :], oacc[0:HD, :])
                        dh = smpool.tile([1, 512], dt.float32, tag="denh", name=f"dh_{i}_{h}")
                        nc.vector.tensor_copy(dh[:], oacc[HD:HD + 1, :])
                        nc.sync.dma_start(den_all[h:h + 1, :], dh[:])
                # batched reciprocal + per-pair broadcast/normalize
                lnd = smpool.tile([8, 512], dt.float32, tag="den2", name=f"lnd_{i}")
                nc.scalar.activation(lnd[:], den_all[:], AF.Ln)
                recip = smpool.tile([8, 512], dt.float32r, tag="den2", name=f"rc_{i}")
                nc.scalar.activation(recip[:], lnd[:], AF.Exp, scale=-1.0)
                for pc in range(CH):
                    rb = ppool.tile([128, 512], dt.float32, tag="sc", name=f"rb_{i}_{pc}")
                    nc.tensor.matmul(rb[:], ind8[:, pc * 128:(pc + 1) * 128],
                                     recip[:], start=True, stop=True)
                    nc.vector.tensor_mul(o_norm[:, pc, :], o_raw[:, pc, :], rb[:])

                # ---- out projection + residual ----
                r1 = rpool.tile([128, CH, NT], dt.float32, tag="resid", name=f"r1_{i}")
                for mo in range(CH):
                    po = ppool.tile([128, 512], dt.float32, tag="mm", name=f"po_{i}_{mo}")
                    for kc in range(CH):
                        nc.tensor.matmul(po[:], w["wo"][:, kc, mo * 128:(mo + 1) * 128],
                                         o_norm[:, kc, :], start=(kc == 0), stop=(kc == CH - 1))
                    nc.vector.scalar_tensor_tensor(
                        r1[:, mo, :], po[:], b_ap(i, 4, mo) if has_out_b else 0.0,
                        q[:, mo, :], AO.add, AO.add)

                # ---- LN1 -> q ----
                _ln(i, 0, r1, q)

                # ---- adaln (ffn) ----
                xf = rpool.tile([128, CH, NT], dt.float32, tag="resid", name=f"xf_{i}")
                xfb = apool.tile([128, CH, NT], dt.bfloat16, tag="act", name=f"xfb_{i}")
                for c in range(CH):
                    nc.vector.tensor_scalar(
                        xf[:, c, :], q[:, c, :],
                        ada[:, ada_col(i, 2, c):ada_col(i, 2, c) + 1],
                        ada[:, ada_col(i, 3, c):ada_col(i, 3, c) + 1],
                        AO.mult, AO.add)
                    nc.scalar.copy(xfb[:, c, :], xf[:, c, :])

                # ---- FFN ----
                hbf = apool.tile([128, CH, NT], dt.bfloat16, tag="act", name=f"hbf_{i}")
                for mh in range(CH):
                    ph = ppool.tile([128, 512], dt.float32, tag="mm", name=f"ph_{i}_{mh}")
                    for kc in range(CH):
                        nc.tensor.matmul(ph[:], w["w1"][:, kc, mh * 128:(mh + 1) * 128],
                                         xfb[:, kc, :], start=(kc == 0), stop=(kc == CH - 1))
                    nc.vector.tensor_scalar(
                        hbf[:, mh, :], ph[:],
                        b_ap(i, 5, mh) if has_b1 else 0.0, 0.0, AO.add, AO.max)
                r2 = rpool.tile([128, CH, NT], dt.float32, tag="resid", name=f"r2_{i}")
                for mo in range(CH):
                    pf = ppool.tile([128, 512], dt.float32, tag="mm", name=f"pf_{i}_{mo}")
                    for kc in range(CH):
                        nc.tensor.matmul(pf[:], w["w2"][:, kc, mo * 128:(mo + 1) * 128],
                                         hbf[:, kc, :], start=(kc == 0), stop=(kc == CH - 1))
                    nc.vector.scalar_tensor_tensor(
                        r2[:, mo, :], pf[:], b_ap(i, 6, mo) if has_b2 else 0.0,
                        xf[:, mo, :], AO.add, AO.add)

                # ---- LN2 -> q ; emit layer output ----
                _ln(i, 1, r2, q)
                for c in range(CH):
                    nc.sync.dma_start(d_out[i, :, c, :], q[:, c, :])

            def _ln(i, which, rin, qout):
                # stats from bf16 copy of rin
                rbf = apool.tile([128, CH, NT], dt.bfloat16, tag="act", name=f"rbf_{i}_{which}")
                r2b = apool.tile([128, CH, NT], dt.bfloat16, tag="act", name=f"r2b_{i}_{which}")
                for c in range(CH):
                    nc.vector.tensor_copy(rbf[:, c, :], rin[:, c, :])
                    nc.vector.tensor_mul(r2b[:, c, :], rbf[:, c, :], rbf[:, c, :])
                pm = ppool.tile([128, 512], dt.float32, tag="mm", name=f"pm_{i}_{which}")
                pv = ppool.tile([128, 512], dt.float32, tag="mm", name=f"pv2_{i}_{which}")
                for c in range(CH):
                    nc.tensor.matmul(pm[0:1, :], ones_sc[:], rbf[:, c, :],
                                     start=(c == 0), stop=(c == CH - 1))
                for c in range(CH):
                    nc.tensor.matmul(pv[0:1, :], ones_sc[:], r2b[:, c, :],
                                     start=(c == 0), stop=(c == CH - 1))
                m_sb = stpool.tile([1, 512], dt.float32r, tag="st", name=f"m_{i}_{which}")
                nc.scalar.copy(m_sb[:], pm[0:1, :])
                msq = stpool.tile([1, 512], dt.float32, tag="st", name=f"msq_{i}_{which}")
                nc.scalar.activation(msq[:], pm[0:1, :], AF.Square)
                var = stpool.tile([1, 512], dt.float32, tag="st", name=f"var_{i}_{which}")
                nc.vector.scalar_tensor_tensor(var[:], pv[0:1, :], 1e-5, msq[:],
                                               AO.add, AO.subtract)
                lnv = stpool.tile([1, 512], dt.float32, tag="st", name=f"lnv_{i}_{which}")
                nc.scalar.activation(lnv[:], var[:], AF.Ln)
                rstd = stpool.tile([1, 512], dt.float32r, tag="st", name=f"rstd_{i}_{which}")
                nc.scalar.activation(rstd[:], lnv[:], AF.Exp, scale=-0.5)
                mb = ppool.tile([128, 512], dt.float32, tag="sc", name=f"mb_{i}_{which}")
                nc.tensor.matmul(mb[:], ones1[:], m_sb[:], start=True, stop=True)
                rsb = ppool.tile([128, 512], dt.float32, tag="sc", name=f"rsb_{i}_{which}")
                nc.tensor.matmul(rsb[:], ones1[:], rstd[:], start=True, stop=True)
                for c in range(CH):
                    t1 = scfpool.tile([128, 512], dt.float32, tag="scf", name=f"lt_{i}_{which}_{c}")
                    nc.vector.tensor_sub(t1[:], rin[:, c, :], mb[:])
                    g_arg = (lng[:, lng_col(i, which * 2, c):lng_col(i, which * 2, c) + 1]
                             if has_ln_g else 1.0)
                    nc.vector.scalar_tensor_tensor(qout[:, c, :], t1[:], g_arg, rsb[:],
                                                   AO.mult, AO.mult)
                    if has_ln_b:
                        nc.vector.tensor_scalar_add(
                            qout[:, c, :], qout[:, c, :],
                            lng[:, lng_col(i, which * 2 + 1, c):lng_col(i, which * 2 + 1, c) + 1])

            if nrep == 1:
                for i in range(L):
                    layer(i)
            else:
                with tc.For_i(0, nrep, 1):
                    for _u in range(unroll):
                        for i in range(L):
                            layer(i)

    nc.compile()
    return nc


def _prep_core(inputs, core, host):
    b, qh = core // 2, core % 2
    sl = slice(qh * NT, (qh + 1) * NT)
    im = {
        "qT": np.ascontiguousarray(_pack_fm(inputs["query"][sl, b, :].T.astype(np.float32))),
        "vT": _pack_fm(inputs["value"][:, b, :].T).astype(BF),
        "cq": _pack_fm(inputs["query_pos"][b, sl, :, 0].T).astype(BF),
        "sq": _pack_fm(host["sgn"] * inputs["query_pos"][b, sl, :, 1].T).astype(BF),
        "ck": _pack_fm(inputs["value_pos"][b, :, :, 0].T).astype(BF),
        "sk": _pack_fm(host["sgn"] * inputs["value_pos"][b, :, :, 1].T).astype(BF),
        "wts": host["wts"],
        "ada": host["ada"][b],
        "ind8": host["ind8"],
        "ones1": np.ones((1, 128), np.float32),
        "ind2": host["ind2"],
        "bias": host["bias"],
        "lng": host["lng"],
    }
    return im


def _prep_host(inputs):
    wts = np.zeros((L, 8, 128, CH, C), BF)
    bias = np.zeros((128, L * 7 * CH), np.float32)
    lng = np.zeros((128, L * 4 * CH), np.float32)
    for i in range(L):
        in_w, in_b = np.asarray(inputs["in_w"][i]), np.asarray(inputs["in_b"][i])
        wq = in_w[:C] * SCALING
        wk, wv = in_w[C:2 * C], in_w[2 * C:]
        bq = in_b[:C] * SCALING
        bk, bv = in_b[C:2 * C], in_b[2 * C:]
        if np.any(bv):
            raise NotImplementedError("nonzero v-projection bias not supported")
        mats = [wk.T, _rot2_rows(wk).T, wv.T, wq.T, _rot2_rows(wq).T,
                np.asarray(inputs["out_w"][i]).T,
                np.asarray(inputs["w1"][i]).T, np.asarray(inputs["w2"][i]).T]
        for j, m in enumerate(mats):
            wts[i, j] = _pack_w(np.ascontiguousarray(m)).astype(BF)
        def _pswap(v):
            o = np.empty_like(v); o[0::2] = v[1::2]; o[1::2] = v[0::2]; return o
        bvecs = [bq, _pswap(bq), bk, _pswap(bk),
                 np.asarray(inputs["out_b"][i]), np.asarray(inputs["b1"][i]),
                 np.asarray(inputs["b2"][i])]
        for qy, v in enumerate(bvecs):
            for c in range(CH):
                bias[:, (i * 7 + qy) * CH + c] = v[c * 128:(c + 1) * 128]
        lvecs = [np.asarray(inputs["ln1_g"][i]), np.asarray(inputs["ln1_b"][i]),
                 np.asarray(inputs["ln2_g"][i]), np.asarray(inputs["ln2_b"][i])]
        for qy, v in enumerate(lvecs):
            for c in range(CH):
                lng[:, (i * 4 + qy) * CH + c] = v[c * 128:(c + 1) * 128]

    ada = np.zeros((4, 128, L * 4 * CH), np.float32)
    diff = np.asarray(inputs["diff_ts"], np.float32)
    for b in range(4):
        st = _silu(diff[b])
        for i in range(L):
            for qy, (aw, ab) in enumerate(
                    [(inputs["aw_attn"][i], inputs["ab_attn"][i]),
                     (inputs["aw_ffn"][i], inputs["ab_ffn"][i])]):
                mod = st @ np.asarray(aw, np.float32).T + np.asarray(ab, np.float32)
                sc, sh = 1.0 + mod[:C], mod[C:]
                for c in range(CH):
                    ada[b, :, (i * 4 + 2 * qy) * CH + c] = sc[c * 128:(c + 1) * 128]
                    ada[b, :, (i * 4 + 2 * qy + 1) * CH + c] = sh[c * 128:(c + 1) * 128]

    ind8 = np.zeros((8, C), np.float32)
    for h in range(H):
        base = (h // 2) * 128 + (h % 2) * 64
        ind8[h, base:base + 64] = 1.0
    ind2 = np.zeros((2, 128), np.float32)
    ind2[0, 0:64] = 1.0
    ind2[1, 64:128] = 1.0
    sgn = np.ones((C, 1), np.float32)
    sgn[0::2] = -1.0

    flags = (bool(np.any(np.asarray(inputs["in_b"]))),
             bool(np.any(np.asarray(inputs["out_b"]))),
             bool(np.any(np.asarray(inputs["b1"]))),
             bool(np.any(np.asarray(inputs["b2"]))),
             bool(np.any(np.asarray(inputs["ln1_g"]) != 1.0) or np.any(np.asarray(inputs["ln2_g"]) != 1.0)),
             bool(np.any(np.asarray(inputs["ln1_b"])) or np.any(np.asarray(inputs["ln2_b"]))))
    return dict(wts=wts, ada=ada, ind8=ind8, ind2=ind2, sgn=sgn, bias=bias, lng=lng), flags


def _get_program(flags, nrep=1, unroll=1):
    key = (flags, nrep, unroll)
    if key not in _CACHE:
        _CACHE[key] = _build(flags, nrep, unroll)
    return _CACHE[key]


def _assemble(results):
    full = np.zeros((L, 1024, 4, C), np.float32)
    for core in range(8):
        b, qh = core // 2, core % 2
        arr = results[core]["out"]                     # [L, 128, CH, NT]
        fm = np.transpose(arr, (0, 2, 1, 3)).reshape(L, C, NT)
        full[:, qh * NT:(qh + 1) * NT, b, :] = np.transpose(fm, (0, 2, 1))
    return full


def kernel(**inputs):
    from concourse.bass_utils import run_bass_kernel_spmd

    inputs = {k: np.asarray(v) for k, v in inputs.items()}
    host, flags = _prep_host(inputs)
    nc = _get_program(flags)
    in_maps = [_prep_core(inputs, core, host) for core in range(8)]
    res = run_bass_kernel_spmd(nc, in_maps, list(range(8)))
    return _assemble(res.results)
